# revision 44
# baseline (speedup 1.0000x reference)
"""Trainium2 Bass kernel for ColorProcessingLossV2 (8-core data-parallel).

Pipeline per core (one batch image, 3x512x1024 pixels laid out as [128, 4096]):
  Both 19-entry tables (NIGHT, CITY) are quantized (9/10 bits) and packed
  into disjoint mantissa bit-fields of a single fp32 value u in [1, 2):
      u = 1 + (a<<14 | b<<2) * 2^-23,  a = quant9(NIGHT), b = quant10(CITY).
  One 9-op DVE staircase builds u' = U[seg]-U[0] exactly; the fused v-pass
  v = chanmax - hi(u)*ALPHA carries a chained min accumulator.  Pre-phase
  chunks are asymmetric [3072, 1024] so the trailing vsplit+vmax tail that
  gates the AllReduce trigger is short.
  The global [vmax, -vmin] AllReduce uses the ncfw mesh collective, preceded
  by TWO dependency-free warm-up collectives emitted at t~0: the first
  absorbs ncfw wake-up, the second aligns the cores so the real AllReduce
  completes in one ~8us mesh round instead of stalling ~20us on peer skew.
  g = ((v-imin)/rng)^0.4 via Ln/Exp on ACT with the baseline's exact
  log-space division (eb = -GAMMA*ln(rng) as the Exp bias); rng/eb scalars
  are computed on DVE so ACT only ever touches the Ln/Exp tables (single
  mid-phase table load).  wc = min(g + city(u) + DELTA_C, 1) is one fused
  DVE op; the three per-channel sums are single-pass fp32
  scalar_tensor_tensor dots with fused accumulation.
  Host reduces the [128, chunks*3] partials in float64.

  (KV3=manual enables an experimental remote_dma_broadcast all-gather that
  replaces the collective; it validates in the multi-core simulator but
  hangs this axon-tunneled hardware, so it is off by default.)
"""

import os
import sys

for _p in ("/root/.axon_site/_ro/trn_rl_repo", "/opt/trn_rl_repo"):
    if _p not in sys.path:
        sys.path.append(_p)

import numpy as np


def _install_ntff_hook_shim():
    """Provide antenv.axon_hooks (NTFF profiling hook) when the image lacks it."""
    try:
        import antenv.axon_hooks  # noqa: F401
        return
    except ImportError:
        pass
    import contextlib
    import ctypes
    import types

    so_path = "/opt/axon/libaxon_pjrt.so"
    state = {"hook": None}

    def _make_hook():
        if not os.path.exists(so_path):
            return None
        lib = ctypes.CDLL(so_path)
        if not hasattr(lib, "axon_start_nrt_profile"):
            return None
        lib.axon_start_nrt_profile.argtypes = [
            ctypes.POINTER(ctypes.c_int64), ctypes.c_size_t]
        lib.axon_start_nrt_profile.restype = ctypes.c_int64
        lib.axon_stop_nrt_profile.argtypes = [ctypes.c_char_p]
        lib.axon_stop_nrt_profile.restype = ctypes.c_int64

        @contextlib.contextmanager
        def _hook(output_dir, device_ids):
            import jax
            jax.devices()
            if device_ids:
                ids = (ctypes.c_int64 * len(device_ids))(*device_ids)
                rc = lib.axon_start_nrt_profile(ids, len(device_ids))
            else:
                rc = lib.axon_start_nrt_profile(None, 0)
            if rc != 0:
                raise RuntimeError(f"axon_start_nrt_profile rc={rc}")
            try:
                yield
            finally:
                n = lib.axon_stop_nrt_profile(str(output_dir).encode())
                print(f"profile: {n} file(s) written to {output_dir}",
                      file=sys.stderr)

        return _hook

    mod = types.ModuleType("antenv.axon_hooks")

    def get_axon_ntff_profile_hook():
        if state["hook"] is None:
            state["hook"] = _make_hook()
        return state["hook"]

    def set_axon_ntff_profile_hook(h):
        state["hook"] = h

    mod.get_axon_ntff_profile_hook = get_axon_ntff_profile_hook
    mod.set_axon_ntff_profile_hook = set_axon_ntff_profile_hook
    sys.modules["antenv.axon_hooks"] = mod


_install_ntff_hook_shim()

import ml_dtypes
import concourse.bacc as bacc
import concourse.bass_isa as bass_isa
import concourse.mybir as mybir
import concourse.tile as tile
from concourse.tile import add_dep_helper
import concourse.dve_ops as dve_ops
from concourse.bass_utils import run_bass_kernel_spmd
from concourse.dve_spec import (
    Spec, Src0, Src1, C0, C1, C2, One, minn, lower, _has_src1)
from concourse.dve_uop import DveOpSpec

BF16 = ml_dtypes.bfloat16

# ---------------------------------------------------------------- constants

_NIGHT = np.array([
    76.5113984140019, 76.23163212875781, 60.90662084364415, 69.06930071129905,
    69.63671393061327, 73.11413822794262, 140.7827781957324, 116.29554873008291,
    46.23329954488532, 57.839322341112386, 32.61465346757989, 57.4385179294615,
    62.234896087294814, 90.90285758569436, 91.99610158117673, 91.82209397173472,
    94.06478985576457, 74.6924145472464, 69.15034088822232], dtype=np.float64) / 255.0
_CITY = np.array([
    86.46051320057052, 79.37014543897092, 95.30679177391578, 71.11888521745776,
    75.57026559270716, 77.90493757655786, 74.77466800282637, 88.27701037425895,
    57.685269557270146, 72.71472387765841, 229.9589238353863, 66.9194012998903,
    60.42471796718752, 76.8407421534007, 74.98657626719087, 73.56771430328095,
    123.92515568872523, 68.93476495876828, 76.0970460111028], dtype=np.float64) / 255.0

_N32 = _NIGHT.astype(np.float32).astype(np.float64)
_C32 = _CITY.astype(np.float32).astype(np.float64)

_NMIN = _N32.min()
_CMIN = _C32.min()
_QA = (_N32.max() - _NMIN) / 511.0
_QB = (_C32.max() - _CMIN) / 1023.0
_AQ = np.round((_N32 - _NMIN) / _QA).astype(np.int64)
_BQ = np.round((_C32 - _CMIN) / _QB).astype(np.int64)
_U_TAB = (1.0 + (_AQ * (1 << 14) + _BQ * (1 << 2)) * 2.0 ** -23).astype(np.float32)
_DU = np.diff(_U_TAB.astype(np.float64))        # exact fp32-representable deltas

SPLIT_S = float(np.float32(3 * 2 ** 13))         # Dekker split constant (24576)
ALPHA = float(np.float32(2 ** 9 * _QA))          # hi -> NIGHT scale
GAMMA_C = float(np.float32(2 ** 21 * _QB))       # (u-hi) -> CITY scale
DELTA_C = float(np.float32(_CMIN + _BQ[0] * _QB))
BIG = 3.0e38

GAMMA = 0.4
LOSS_WEIGHT = 1.0

# ablation flags (comma list in KV3):
#   ccar   - use ncfw collective_compute AllReduce instead of manual p2p
_ABL = set(filter(None, os.environ.get("KV3", "").split(",")))
MANUAL_AR = "manual" in _ABL

B, C, H, W_IMG = 8, 3, 512, 1024
P = 128
F = (H * W_IMG) // P          # 4096 free elements per partition
N_CORES = 8

# ---------------------------------------------------------------- custom ops


def _register_dve_op(name, spec, subdim=False):
    for op in dve_ops.OPS:
        if op.name == name:
            return op
    row = dve_ops._CUSTOM_DVE_ROW_BASE + len(dve_ops.OPS)
    assert row < 0x20, "custom DVE opcode rows exhausted"
    shas = {}
    for ver in ("v3", "v4"):
        tmp = DveOpSpec(name=name, opcode=row, uops=lower(spec, ver=ver),
                        rd1_en=_has_src1(spec))
        shas[ver] = tmp.sha(ver)
    op = dve_ops.DveOp(name, spec, subdim=subdim, uops_sha=shas)
    dve_ops.OPS.append(op)
    dve_ops.CUSTOM_DVE_SPECS[name] = spec
    dve_ops._SUB_OPCODE_FOR_NAME[name] = row
    return op


def _stair2_ref(in0, in1, s0, s1, imm2):
    x = in0.astype(np.float32)
    return (in1 + (x >= s0) * np.float32(s1) + (x > s0) * np.float32(imm2)
            ).astype(np.float32)


# out = acc + (seg >= k) * d0 + (seg > k) * d1   (two staircase steps)
STAIR2 = _register_dve_op(
    "STAIR2_ANT",
    Spec(body=Src1 + (Src0 >= C0) * C1 + (Src0 > C0) * C2,
         reference=_stair2_ref),
)


def _stair2_init_ref(in0, in1, s0, s1, imm2):
    x = in0.astype(np.float32)
    return ((x >= s0) * np.float32(s1) + (x > s0) * np.float32(imm2)
            ).astype(np.float32)


# first staircase op of a zero-seeded chain (no accumulator input).
STAIR2_INIT = _register_dve_op(
    "STAIR2_INIT_ANT",
    Spec(body=(Src0 >= C0) * C1 + (Src0 > C0) * C2,
         reference=_stair2_init_ref),
)


def _vsplit_ref(in0, in1, s0, s1, imm2):
    f = np.float32
    t = f(in1.astype(f) + f(s1))
    hi = f(t - f(s1))
    out = f(in0.astype(f) - f(hi * f(imm2)))
    acc = np.minimum(
        np.minimum.reduce(out.reshape(out.shape[0], -1), axis=-1,
                          keepdims=True),
        np.asarray(s0, f).reshape(-1, 1))
    return out, acc.astype(f)


# v = cmax - hi(u)*alpha ; accum_out = min(seed, min(v)); seed is a [P,1] AP
_hi_v = (Src1 + C1) - C1
VSPLIT_AMIN = _register_dve_op(
    "VSPLIT_AMIN_ANT",
    Spec(body=Src0 - _hi_v * C2, accum=minn, accum_init=C0,
         reference=_vsplit_ref),
)


def _wsplitc_ref(in0, in1, s0, s1, imm2):
    f = np.float32
    t = f(in1.astype(f) + f(s0))
    hi = f(t - f(s0))
    d = f(in1.astype(f) - hi)
    return np.minimum(f(f(in0.astype(f) + f(d * f(s1))) + f(imm2)), f(1.0))


# wc = min(g + (u - hi(u))*gamma_c + delta_c, 1)
_hi_w = (Src1 + C0) - C0
WSPLITC = _register_dve_op(
    "WSPLITC_ANT",
    Spec(body=minn(Src0 + (Src1 - _hi_w) * C1 + C2, One),
         reference=_wsplitc_ref),
)

# ---------------------------------------------------------------- bass build

_COMPILED = None

CW = 2048
SLICES = [slice(0, CW), slice(CW, F)]
N_CH = len(SLICES)
# asymmetric pre-phase chunks: a short trailing chunk shrinks the
# vsplit+vmax tail that gates the AllReduce trigger
PRE_SLICES = [slice(0, 3072), slice(3072, F)]


def _build():
    dt = mybir.dt
    alu = mybir.AluOpType
    nc = bacc.Bacc("TRN2", target_bir_lowering=False, debug=False,
                   num_devices=N_CORES)
    org_d = nc.dram_tensor("org", [C, P, F], dt.bfloat16, kind="ExternalInput").ap()
    seg_d = nc.dram_tensor("seg", [P, F], dt.bfloat16, kind="ExternalInput").ap()
    refl_d = nc.dram_tensor("refl", [C, P, F], dt.float32, kind="ExternalInput").ap()
    out_d = nc.dram_tensor("out", [P, 3 * N_CH + 4], dt.float32,
                           kind="ExternalOutput").ap()

    # raw (untracked) gather buffer for the manual all-reduce: 8 slots of
    # per-partition [vmax, -vmin]; peers remote-write into their rank's slot.
    gbuf = nc.alloc_sbuf_tensor("argather", [P, 2 * N_CORES], dt.float32).ap()
    ar_ms = nc.monotonic_semaphore(0)
    ar_rsem = ar_ms.sem()
    ar_lsem = nc.alloc_semaphore("ar_lsem")
    # expected-count register write lands in the pre-Tile preamble; the wait
    # itself is attached to the consuming instruction below so the Tile
    # scheduler cannot hoist it.
    ar_ms.inc_expected(2 * N_CORES)

    with tile.TileContext(nc) as tc:
        with (
            tc.tile_pool(name="pers", bufs=1) as pers,
            tc.tile_pool(name="work", bufs=2) as work,
            tc.tile_pool(name="orgp", bufs=1) as orgp,
            tc.tile_pool(name="accp", bufs=8) as accp,
            tc.tile_pool(name="dram", bufs=1, space="DRAM") as dram,
        ):
            if not MANUAL_AR:
                # two warm-up collectives with NO input dependencies (the
                # reduced values are never read), emitted first so the ncfw
                # wake-up starts at t~0; the second round aligns the cores
                # so the real AllReduce sees minimal peer skew
                for wu in range(2):
                    wu_i = dram.tile([1, 2], dt.float32, name=f"wui{wu}")
                    wu_o = dram.tile([1, 2], dt.float32, addr_space="Shared",
                                     name=f"wuo{wu}")
                    nc.gpsimd.collective_compute(
                        "AllReduce", mybir.AluOpType.max,
                        replica_groups=[list(range(N_CORES))],
                        ins=[wu_i.opt()], outs=[wu_o.opt()])

            seg = pers.tile([P, F], dt.bfloat16, tag="seg")
            u = pers.tile([P, F], dt.float32, tag="u")
            v = pers.tile([P, F], dt.float32, tag="v")
            rfs = {}
            for c in range(C):
                rf = pers.tile([P, F], dt.float32, tag=f"rf{c}")
                rfs[c] = rf

            # seed for the chained min accumulator
            mseed = pers.tile([P, 1], dt.float32, tag="mseed")
            nc.gpsimd.memset(mseed[:], BIG)
            # ACT table prewarm (Ln) on a dummy [P,1] tile
            dmy = pers.tile([P, 1], dt.float32, tag="dmy")
            nc.gpsimd.memset(dmy[:], 1.0)
            nc.scalar.activation(dmy[:], dmy[:],
                                 mybir.ActivationFunctionType.Ln)

            mm = pers.tile([P, 2], dt.float32, tag="mm")
            mmr = pers.tile([P, 2], dt.float32, tag="mmr")
            gprr = pers.tile([P, 2 * N_CORES], dt.float32, tag="gprr")
            gmm = pers.tile([P, 2], dt.float32, tag="gmm")

            # ---- input DMA (sync/SP queue) ----
            for sl in PRE_SLICES:
                nc.sync.dma_start(seg[:, sl], seg_d[:, sl])
            orgs = {}
            for ch, sl in enumerate(PRE_SLICES):
                cw = sl.stop - sl.start
                for c in range(C):
                    o = orgp.tile([P, cw], dt.bfloat16, tag=f"org{c}",
                                  name=f"org{c}_{ch}")
                    nc.sync.dma_start(o[:], org_d[c, :, sl])
                    orgs[(ch, c)] = o
            for c in range(C):
                nc.sync.dma_start(rfs[c][:], refl_d[c])

            _ctr = [0]

            def wt(n, dtype=dt.float32, tag="buf"):
                _ctr[0] += 1
                return work.tile([P, n], dtype, tag=tag,
                                 name=f"wb{_ctr[0]}")

            # ---- pre-phase per chunk ----
            mn_prev = mseed
            mxs = []
            for ch, sl in enumerate(PRE_SLICES):
                cw = sl.stop - sl.start
                # chanmax (bf16, 2x DVE)
                t01 = orgp.tile([P, cw], dt.bfloat16, tag="t01",
                                name=f"t01_{ch}")
                cmx = orgp.tile([P, cw], dt.bfloat16, tag="cmx",
                                name=f"cmx_{ch}")
                nc.vector.tensor_tensor(t01[:], orgs[(ch, 0)][:],
                                        orgs[(ch, 1)][:], alu.max)
                nc.vector.tensor_tensor(cmx[:], t01[:], orgs[(ch, 2)][:],
                                        alu.max)
                # 9-op staircase chain
                sa, sb = wt(cw, tag="sa"), wt(cw, tag="sb")
                cur = None
                for j in range(9):
                    k = 1 + 2 * j
                    outt = u[:, sl] if j == 8 else (sa[:] if j % 2 == 0 else sb[:])
                    if cur is None:
                        nc.vector._custom_dve(
                            STAIR2_INIT, out=outt, in0=seg[:, sl],
                            s0=float(k), s1=float(_DU[k - 1]),
                            imm2=float(_DU[k]))
                    else:
                        nc.vector._custom_dve(
                            STAIR2, out=outt, in0=seg[:, sl], in1=cur,
                            s0=float(k), s1=float(_DU[k - 1]),
                            imm2=float(_DU[k]))
                    cur = outt
                # v-pass with chained min accumulator
                mn = pers.tile([P, 1], dt.float32, tag=f"mn{ch}")
                nc.vector._custom_dve(
                    VSPLIT_AMIN, out=v[:, sl], in0=cmx[:],
                    in1=u[:, sl], s0=mn_prev[:, 0:1], s1=SPLIT_S,
                    imm2=ALPHA, accum_out=mn[:])
                mn_prev = mn
                mx = pers.tile([P, 1], dt.float32, tag=f"mx{ch}")
                nc.vector.tensor_reduce(mx[:], v[:, sl], mybir.AxisListType.X,
                                        alu.max)
                mxs.append(mx)

            # ---- per-core per-partition [vmax, -vmin] ----
            nc.vector.tensor_tensor(mm[:, 0:1], mxs[0][:], mxs[1][:], alu.max)
            mm_done = nc.vector.tensor_scalar(mm[:, 1:2], mn_prev[:], -1.0,
                                              None, alu.mult)

            # only the DIFFERENCES of the channel means enter the loss, so
            # two dots suffice; the channel-difference subtracts are
            # AllReduce-independent and fill the otherwise idle DVE window
            # (in place: rf1 <- rf0-rf1, rf2 <- rf0-rf2).  The explicit dep
            # on the mm build keeps the Tile scheduler from hoisting them
            # into the pre-phase, which would delay the AllReduce trigger.
            s1 = nc.vector.tensor_tensor(rfs[1][:], rfs[0][:], rfs[1][:],
                                         alu.subtract)
            add_dep_helper(s1.ins, mm_done.ins,
                           reason="keep channel-diff in the AllReduce shadow")
            s2 = nc.vector.tensor_tensor(rfs[2][:], rfs[0][:], rfs[2][:],
                                         alu.subtract)
            add_dep_helper(s2.ins, mm_done.ins,
                           reason="keep channel-diff in the AllReduce shadow")

            if MANUAL_AR:
                # each core broadcasts its [128,2] into its rank's slot of
                # every core's gather buffer, then waits for all 8
                rank = nc.gpsimd.partition_id()
                rdests = [(0, k) for k in range(N_CORES)]
                for r in tc.Switch(rank, N_CORES, hint="ar_slot"):
                    nc.gpsimd.remote_dma_broadcast(
                        gbuf[:, 2 * r:2 * r + 2], mm[:],
                        remote_sem=ar_rsem, local_sem=ar_lsem,
                        rdests=rdests)
                    nc.gpsimd.trigger_dma(count=None)
                # cross-partition reduce of all 8 slots (tracked output).
                # The register-valued semaphore wait (16 = 8 senders x 2
                # remote increments) is attached to this instruction's
                # sync_info so the Tile scheduler cannot hoist it; the
                # scheduling sim treats register waits as satisfiable.
                par = nc.gpsimd.partition_all_reduce(
                    gprr[:], gbuf, channels=P,
                    reduce_op=bass_isa.ReduceOp.max)
                par.wait_op(ar_rsem, ar_ms._reg, "sem-ge")
                # 8 -> 4 -> 2 -> 1 slot max on DVE (tracked gprr input)
                gv = gprr[:].rearrange("p (s two) -> p s two", two=2)
                g4 = pers.tile([P, 8], dt.float32, tag="g4")
                g4v = g4[:].rearrange("p (s two) -> p s two", two=2)
                nc.vector.tensor_tensor(g4v, gv[:, 0:4], gv[:, 4:8], alu.max)
                g2 = pers.tile([P, 4], dt.float32, tag="g2")
                g2v = g2[:].rearrange("p (s two) -> p s two", two=2)
                nc.vector.tensor_tensor(g2v, g4v[:, 0:2], g4v[:, 2:4],
                                        alu.max)
                nc.vector.tensor_tensor(gmm[:], g2[:, 0:2], g2[:, 2:4],
                                        alu.max)
            else:
                flat = pers.tile([1, 2 * P], dt.float32, tag="flat")
                nc.sync.dma_start(flat[:], mm[:])
                l1 = pers.tile([1, 2], dt.float32, tag="l1")
                nc.vector.tensor_reduce(
                    l1[:], flat[:].rearrange("p (c two) -> p two c", two=2),
                    mybir.AxisListType.X, alu.max)
                agi = dram.tile([1, 2], dt.float32)
                aro = dram.tile([1, 2], dt.float32, addr_space="Shared")
                nc.sync.dma_start(agi[:], l1[:])
                nc.gpsimd.collective_compute(
                    "AllReduce", alu.max,
                    replica_groups=[list(range(N_CORES))],
                    ins=[agi.opt()], outs=[aro.opt()])
                nc.sync.dma_start(gmm[:], aro.opt().partition_broadcast(P))

            # debug columns: gmm and mm
            nc.sync.dma_start(out_d[:, 6:8], gmm[:])
            nc.sync.dma_start(out_d[:, 8:10], mm[:])

            # negmin = gmm[:,1:2]; divide by rng in log space (exactly the
            # proven baseline numerics): g = exp(GAMMA*ln(v-imin) + eb),
            # eb = -GAMMA*ln(rng).  rng/eb arithmetic on DVE so ACT only
            # ever touches the Ln/Exp tables.
            negmin = gmm[:, 1:2]
            rng_t = pers.tile([P, 1], dt.float32, tag="rng")
            nc.vector.tensor_tensor(rng_t[:], gmm[:, 0:1], gmm[:, 1:2],
                                    alu.add)
            lnr = pers.tile([P, 1], dt.float32, tag="lnr")
            nc.scalar.activation(lnr[:], rng_t[:],
                                 mybir.ActivationFunctionType.Ln)
            eb = pers.tile([P, 1], dt.float32, tag="eb")
            nc.vector.tensor_scalar(eb[:], lnr[:], -GAMMA, None, alu.mult)

            # ---- post-phase per chunk ----
            for pc, sl in enumerate(SLICES):
                nc.scalar.activation(v[:, sl], v[:, sl],
                                     mybir.ActivationFunctionType.Ln,
                                     bias=negmin, scale=1.0)
            for pc, sl in enumerate(SLICES):
                cw = sl.stop - sl.start
                nc.scalar.activation(v[:, sl], v[:, sl],
                                     mybir.ActivationFunctionType.Exp,
                                     bias=eb[:, 0:1], scale=GAMMA)
                wc = wt(cw, tag="wc")
                nc.vector._custom_dve(
                    WSPLITC, out=wc[:], in0=v[:, sl], in1=u[:, sl],
                    s0=SPLIT_S, s1=GAMMA_C, imm2=DELTA_C)
                dmp = wt(cw, tag="dmp")
                for j, c in enumerate((1, 2)):
                    accn = accp.tile([P, 1], dt.float32, tag="accs",
                                     name=f"acc{pc}_{c}")
                    nc.vector.scalar_tensor_tensor(
                        dmp[:], wc[:], 0.0, rfs[c][:, sl],
                        alu.add, alu.mult, accum_out=accn[:])
                    nc.sync.dma_start(
                        out_d[:, (pc * 2 + j):(pc * 2 + j + 1)], accn[:])

    nc.compile()
    return nc


def _get_compiled():
    global _COMPILED
    if _COMPILED is None:
        _COMPILED = _build()
    return _COMPILED


# ---------------------------------------------------------------- entry point

def kernel(reflectance, org_img, seg_label, _trace=False):
    reflectance = np.ascontiguousarray(np.asarray(reflectance, dtype=np.float32))
    org_img = np.asarray(org_img, dtype=np.float32).astype(BF16)
    seg_label = np.asarray(seg_label).astype(BF16)

    nc = _get_compiled()
    in_maps = []
    for i in range(N_CORES):
        in_maps.append({
            "org": np.ascontiguousarray(org_img[i].reshape(C, P, F)),
            "seg": np.ascontiguousarray(seg_label[i].reshape(P, F)),
            "refl": np.ascontiguousarray(reflectance[i].reshape(C, P, F)),
        })
    res = run_bass_kernel_spmd(nc, in_maps, core_ids=list(range(N_CORES)),
                               trace=_trace)
    totals = np.zeros(2, dtype=np.float64)
    for i in range(N_CORES):
        o = res.results[i]["out"].astype(np.float64)   # [128, N_CH x 2 + 6]
        if os.environ.get("KV3_DBG"):
            print(f"core {i}: gmm={o[0, 6:8]} gmm[64]={o[64, 6:8]} "
                  f"mm[0]={o[0, 8:10]} mm[64]={o[64, 8:10]}")
        totals += o[:, :4].sum(axis=0).reshape(N_CH, 2).sum(axis=0)
    # accs hold sum(wc*(r0-r1)) and sum(wc*(r0-r2))
    d01, d02 = totals / float(B * H * W_IMG)
    loss = LOSS_WEIGHT * (d01 ** 2 + d02 ** 2 + (d02 - d01) ** 2)
    if _trace:
        kernel._last_exec_time_ns = res.exec_time_ns
        kernel._last_results = res
    return np.float32(loss)


# revision 46
# speedup vs baseline: 1.0278x; 1.0278x over previous
"""Trainium2 Bass kernel for ColorProcessingLossV2 (8-core data-parallel).

Pipeline per core (one batch image, 3x512x1024 pixels laid out as [128, 4096]):
  Both 19-entry tables (NIGHT, CITY) are quantized (9/10 bits) and packed
  into disjoint mantissa bit-fields of a single fp32 value u in [1, 2):
      u = 1 + (a<<14 | b<<2) * 2^-23,  a = quant9(NIGHT), b = quant10(CITY).
  One 9-op DVE staircase builds u' = U[seg]-U[0] exactly; the fused v-pass
  v = chanmax - hi(u)*ALPHA carries a chained min accumulator.  Pre-phase
  chunks are asymmetric [3072, 1024] so the trailing vsplit+vmax tail that
  gates the AllReduce trigger is short.
  The global [vmax, -vmin] AllReduce uses the ncfw mesh collective, preceded
  by TWO dependency-free warm-up collectives emitted at t~0: the first
  absorbs ncfw wake-up, the second aligns the cores so the real AllReduce
  completes in one ~8us mesh round instead of stalling ~20us on peer skew.
  g = ((v-imin)/rng)^0.4 via Ln/Exp on ACT with the baseline's exact
  log-space division (eb = -GAMMA*ln(rng) as the Exp bias); rng/eb scalars
  are computed on DVE so ACT only ever touches the Ln/Exp tables (single
  mid-phase table load).  wc = min(g + city(u) + DELTA_C, 1) is one fused
  DVE op; the three per-channel sums are single-pass fp32
  scalar_tensor_tensor dots with fused accumulation.
  Host reduces the [128, chunks*3] partials in float64.

  (KV3=manual enables an experimental remote_dma_broadcast all-gather that
  replaces the collective; it validates in the multi-core simulator but
  hangs this axon-tunneled hardware, so it is off by default.)
"""

import os
import sys

for _p in ("/root/.axon_site/_ro/trn_rl_repo", "/opt/trn_rl_repo"):
    if _p not in sys.path:
        sys.path.append(_p)

import numpy as np


def _install_ntff_hook_shim():
    """Provide antenv.axon_hooks (NTFF profiling hook) when the image lacks it."""
    try:
        import antenv.axon_hooks  # noqa: F401
        return
    except ImportError:
        pass
    import contextlib
    import ctypes
    import types

    so_path = "/opt/axon/libaxon_pjrt.so"
    state = {"hook": None}

    def _make_hook():
        if not os.path.exists(so_path):
            return None
        lib = ctypes.CDLL(so_path)
        if not hasattr(lib, "axon_start_nrt_profile"):
            return None
        lib.axon_start_nrt_profile.argtypes = [
            ctypes.POINTER(ctypes.c_int64), ctypes.c_size_t]
        lib.axon_start_nrt_profile.restype = ctypes.c_int64
        lib.axon_stop_nrt_profile.argtypes = [ctypes.c_char_p]
        lib.axon_stop_nrt_profile.restype = ctypes.c_int64

        @contextlib.contextmanager
        def _hook(output_dir, device_ids):
            import jax
            jax.devices()
            if device_ids:
                ids = (ctypes.c_int64 * len(device_ids))(*device_ids)
                rc = lib.axon_start_nrt_profile(ids, len(device_ids))
            else:
                rc = lib.axon_start_nrt_profile(None, 0)
            if rc != 0:
                raise RuntimeError(f"axon_start_nrt_profile rc={rc}")
            try:
                yield
            finally:
                n = lib.axon_stop_nrt_profile(str(output_dir).encode())
                print(f"profile: {n} file(s) written to {output_dir}",
                      file=sys.stderr)

        return _hook

    mod = types.ModuleType("antenv.axon_hooks")

    def get_axon_ntff_profile_hook():
        if state["hook"] is None:
            state["hook"] = _make_hook()
        return state["hook"]

    def set_axon_ntff_profile_hook(h):
        state["hook"] = h

    mod.get_axon_ntff_profile_hook = get_axon_ntff_profile_hook
    mod.set_axon_ntff_profile_hook = set_axon_ntff_profile_hook
    sys.modules["antenv.axon_hooks"] = mod


_install_ntff_hook_shim()

import ml_dtypes
import concourse.bacc as bacc
import concourse.bass_isa as bass_isa
import concourse.mybir as mybir
import concourse.tile as tile
from concourse.tile import add_dep_helper
import concourse.dve_ops as dve_ops
from concourse.bass_utils import run_bass_kernel_spmd
from concourse.dve_spec import (
    Spec, Src0, Src1, C0, C1, C2, One, minn, lower, _has_src1)
from concourse.dve_uop import DveOpSpec

BF16 = ml_dtypes.bfloat16

# ---------------------------------------------------------------- constants

_NIGHT = np.array([
    76.5113984140019, 76.23163212875781, 60.90662084364415, 69.06930071129905,
    69.63671393061327, 73.11413822794262, 140.7827781957324, 116.29554873008291,
    46.23329954488532, 57.839322341112386, 32.61465346757989, 57.4385179294615,
    62.234896087294814, 90.90285758569436, 91.99610158117673, 91.82209397173472,
    94.06478985576457, 74.6924145472464, 69.15034088822232], dtype=np.float64) / 255.0
_CITY = np.array([
    86.46051320057052, 79.37014543897092, 95.30679177391578, 71.11888521745776,
    75.57026559270716, 77.90493757655786, 74.77466800282637, 88.27701037425895,
    57.685269557270146, 72.71472387765841, 229.9589238353863, 66.9194012998903,
    60.42471796718752, 76.8407421534007, 74.98657626719087, 73.56771430328095,
    123.92515568872523, 68.93476495876828, 76.0970460111028], dtype=np.float64) / 255.0

_N32 = _NIGHT.astype(np.float32).astype(np.float64)
_C32 = _CITY.astype(np.float32).astype(np.float64)

_NMIN = _N32.min()
_CMIN = _C32.min()
_QA = (_N32.max() - _NMIN) / 511.0
_QB = (_C32.max() - _CMIN) / 1023.0
_AQ = np.round((_N32 - _NMIN) / _QA).astype(np.int64)
_BQ = np.round((_C32 - _CMIN) / _QB).astype(np.int64)
_U_TAB = (1.0 + (_AQ * (1 << 14) + _BQ * (1 << 2)) * 2.0 ** -23).astype(np.float32)
_DU = np.diff(_U_TAB.astype(np.float64))        # exact fp32-representable deltas

SPLIT_S = float(np.float32(3 * 2 ** 13))         # Dekker split constant (24576)
ALPHA = float(np.float32(2 ** 9 * _QA))          # hi -> NIGHT scale
GAMMA_C = float(np.float32(2 ** 21 * _QB))       # (u-hi) -> CITY scale
DELTA_C = float(np.float32(_CMIN + _BQ[0] * _QB))
BIG = 3.0e38

GAMMA = 0.4
LOSS_WEIGHT = 1.0

# ablation flags (comma list in KV3):
#   ccar   - use ncfw collective_compute AllReduce instead of manual p2p
_ABL = set(filter(None, os.environ.get("KV3", "").split(",")))
MANUAL_AR = "manual" in _ABL

B, C, H, W_IMG = 8, 3, 512, 1024
P = 128
F = (H * W_IMG) // P          # 4096 free elements per partition
N_CORES = 8

# ---------------------------------------------------------------- custom ops


def _register_dve_op(name, spec, subdim=False):
    for op in dve_ops.OPS:
        if op.name == name:
            return op
    row = dve_ops._CUSTOM_DVE_ROW_BASE + len(dve_ops.OPS)
    assert row < 0x20, "custom DVE opcode rows exhausted"
    shas = {}
    for ver in ("v3", "v4"):
        tmp = DveOpSpec(name=name, opcode=row, uops=lower(spec, ver=ver),
                        rd1_en=_has_src1(spec))
        shas[ver] = tmp.sha(ver)
    op = dve_ops.DveOp(name, spec, subdim=subdim, uops_sha=shas)
    dve_ops.OPS.append(op)
    dve_ops.CUSTOM_DVE_SPECS[name] = spec
    dve_ops._SUB_OPCODE_FOR_NAME[name] = row
    return op


def _stair2_ref(in0, in1, s0, s1, imm2):
    x = in0.astype(np.float32)
    return (in1 + (x >= s0) * np.float32(s1) + (x > s0) * np.float32(imm2)
            ).astype(np.float32)


# out = acc + (seg >= k) * d0 + (seg > k) * d1   (two staircase steps)
STAIR2 = _register_dve_op(
    "STAIR2_ANT",
    Spec(body=Src1 + (Src0 >= C0) * C1 + (Src0 > C0) * C2,
         reference=_stair2_ref),
)


def _stair2_init_ref(in0, in1, s0, s1, imm2):
    x = in0.astype(np.float32)
    return ((x >= s0) * np.float32(s1) + (x > s0) * np.float32(imm2)
            ).astype(np.float32)


# first staircase op of a zero-seeded chain (no accumulator input).
STAIR2_INIT = _register_dve_op(
    "STAIR2_INIT_ANT",
    Spec(body=(Src0 >= C0) * C1 + (Src0 > C0) * C2,
         reference=_stair2_init_ref),
)


def _vsplit_ref(in0, in1, s0, s1, imm2):
    f = np.float32
    t = f(in1.astype(f) + f(s1))
    hi = f(t - f(s1))
    out = f(in0.astype(f) - f(hi * f(imm2)))
    acc = np.minimum(
        np.minimum.reduce(out.reshape(out.shape[0], -1), axis=-1,
                          keepdims=True),
        np.asarray(s0, f).reshape(-1, 1))
    return out, acc.astype(f)


# v = cmax - hi(u)*alpha ; accum_out = min(seed, min(v)); seed is a [P,1] AP
_hi_v = (Src1 + C1) - C1
VSPLIT_AMIN = _register_dve_op(
    "VSPLIT_AMIN_ANT",
    Spec(body=Src0 - _hi_v * C2, accum=minn, accum_init=C0,
         reference=_vsplit_ref),
)


def _wsplitc_ref(in0, in1, s0, s1, imm2):
    f = np.float32
    t = f(in1.astype(f) + f(s0))
    hi = f(t - f(s0))
    d = f(in1.astype(f) - hi)
    return np.minimum(f(f(in0.astype(f) + f(d * f(s1))) + f(imm2)), f(1.0))


# wc = min(g + (u - hi(u))*gamma_c + delta_c, 1)
_hi_w = (Src1 + C0) - C0
WSPLITC = _register_dve_op(
    "WSPLITC_ANT",
    Spec(body=minn(Src0 + (Src1 - _hi_w) * C1 + C2, One),
         reference=_wsplitc_ref),
)

# ---------------------------------------------------------------- bass build

_COMPILED = None

CW = 2048
SLICES = [slice(0, CW), slice(CW, F)]
N_CH = len(SLICES)
# asymmetric pre-phase chunks: a short trailing chunk shrinks the
# vsplit+vmax tail that gates the AllReduce trigger
PRE_SLICES = [slice(0, 3072), slice(3072, F)]
# asymmetric post chunks: the small leading chunk lets the DVE
# wsplitc/dot chain start as soon as the first short EXP finishes
POST_SLICES = [slice(0, 1024), slice(1024, F)]


def _build():
    dt = mybir.dt
    alu = mybir.AluOpType
    nc = bacc.Bacc("TRN2", target_bir_lowering=False, debug=False,
                   num_devices=N_CORES)
    org_d = nc.dram_tensor("org", [C, P, F], dt.bfloat16, kind="ExternalInput").ap()
    seg_d = nc.dram_tensor("seg", [P, F], dt.bfloat16, kind="ExternalInput").ap()
    refl_d = nc.dram_tensor("refl", [C, P, F], dt.float32, kind="ExternalInput").ap()
    out_d = nc.dram_tensor("out", [P, 3 * N_CH + 4], dt.float32,
                           kind="ExternalOutput").ap()

    # raw (untracked) gather buffer for the manual all-reduce: 8 slots of
    # per-partition [vmax, -vmin]; peers remote-write into their rank's slot.
    gbuf = nc.alloc_sbuf_tensor("argather", [P, 2 * N_CORES], dt.float32).ap()
    ar_ms = nc.monotonic_semaphore(0)
    ar_rsem = ar_ms.sem()
    ar_lsem = nc.alloc_semaphore("ar_lsem")
    # expected-count register write lands in the pre-Tile preamble; the wait
    # itself is attached to the consuming instruction below so the Tile
    # scheduler cannot hoist it.
    ar_ms.inc_expected(2 * N_CORES)

    with tile.TileContext(nc) as tc:
        with (
            tc.tile_pool(name="pers", bufs=1) as pers,
            tc.tile_pool(name="work", bufs=1) as work,
            tc.tile_pool(name="wcp", bufs=2) as wcp,
            tc.tile_pool(name="orgp", bufs=1) as orgp,
            tc.tile_pool(name="accp", bufs=8) as accp,
            tc.tile_pool(name="dram", bufs=1, space="DRAM") as dram,
        ):
            if not MANUAL_AR:
                # two warm-up collectives with NO input dependencies (the
                # reduced values are never read), emitted first so the ncfw
                # wake-up starts at t~0; the second round aligns the cores
                # so the real AllReduce sees minimal peer skew
                for wu in range(2):
                    wu_i = dram.tile([1, 2], dt.float32, name=f"wui{wu}")
                    wu_o = dram.tile([1, 2], dt.float32, addr_space="Shared",
                                     name=f"wuo{wu}")
                    nc.gpsimd.collective_compute(
                        "AllReduce", mybir.AluOpType.max,
                        replica_groups=[list(range(N_CORES))],
                        ins=[wu_i.opt()], outs=[wu_o.opt()])

            seg = pers.tile([P, F], dt.bfloat16, tag="seg")
            u = pers.tile([P, F], dt.float32, tag="u")
            v = pers.tile([P, F], dt.float32, tag="v")
            rfs = {}
            for c in range(C):
                rf = pers.tile([P, F], dt.float32, tag=f"rf{c}")
                rfs[c] = rf

            # seed for the chained min accumulator
            mseed = pers.tile([P, 1], dt.float32, tag="mseed")
            nc.gpsimd.memset(mseed[:], BIG)
            # ACT table prewarm (Ln) on a dummy [P,1] tile
            dmy = pers.tile([P, 1], dt.float32, tag="dmy")
            nc.gpsimd.memset(dmy[:], 1.0)
            nc.scalar.activation(dmy[:], dmy[:],
                                 mybir.ActivationFunctionType.Ln)

            mm = pers.tile([P, 2], dt.float32, tag="mm")
            mmr = pers.tile([P, 2], dt.float32, tag="mmr")
            gprr = pers.tile([P, 2 * N_CORES], dt.float32, tag="gprr")
            gmm = pers.tile([P, 2], dt.float32, tag="gmm")

            # ---- input DMA (sync/SP queue) ----
            for sl in PRE_SLICES:
                nc.sync.dma_start(seg[:, sl], seg_d[:, sl])
            orgs = {}
            for ch, sl in enumerate(PRE_SLICES):
                cw = sl.stop - sl.start
                for c in range(C):
                    o = orgp.tile([P, cw], dt.bfloat16, tag=f"org{c}",
                                  name=f"org{c}_{ch}")
                    nc.sync.dma_start(o[:], org_d[c, :, sl])
                    orgs[(ch, c)] = o
            for c in range(C):
                nc.sync.dma_start(rfs[c][:], refl_d[c])

            _ctr = [0]

            def wt(n, dtype=dt.float32, tag="buf"):
                _ctr[0] += 1
                return work.tile([P, n], dtype, tag=tag,
                                 name=f"wb{_ctr[0]}")

            # ---- pre-phase per chunk ----
            mn_prev = mseed
            mxs = []
            for ch, sl in enumerate(PRE_SLICES):
                cw = sl.stop - sl.start
                # chanmax (bf16, 2x DVE)
                t01 = orgp.tile([P, cw], dt.bfloat16, tag="t01",
                                name=f"t01_{ch}")
                cmx = orgp.tile([P, cw], dt.bfloat16, tag="cmx",
                                name=f"cmx_{ch}")
                nc.vector.tensor_tensor(t01[:], orgs[(ch, 0)][:],
                                        orgs[(ch, 1)][:], alu.max)
                nc.vector.tensor_tensor(cmx[:], t01[:], orgs[(ch, 2)][:],
                                        alu.max)
                # 9-op staircase chain
                sa, sb = wt(cw, tag="sa"), wt(cw, tag="sb")
                cur = None
                for j in range(9):
                    k = 1 + 2 * j
                    outt = u[:, sl] if j == 8 else (sa[:] if j % 2 == 0 else sb[:])
                    if cur is None:
                        nc.vector._custom_dve(
                            STAIR2_INIT, out=outt, in0=seg[:, sl],
                            s0=float(k), s1=float(_DU[k - 1]),
                            imm2=float(_DU[k]))
                    else:
                        nc.vector._custom_dve(
                            STAIR2, out=outt, in0=seg[:, sl], in1=cur,
                            s0=float(k), s1=float(_DU[k - 1]),
                            imm2=float(_DU[k]))
                    cur = outt
                # v-pass with chained min accumulator
                mn = pers.tile([P, 1], dt.float32, tag=f"mn{ch}")
                nc.vector._custom_dve(
                    VSPLIT_AMIN, out=v[:, sl], in0=cmx[:],
                    in1=u[:, sl], s0=mn_prev[:, 0:1], s1=SPLIT_S,
                    imm2=ALPHA, accum_out=mn[:])
                mn_prev = mn
                mx = pers.tile([P, 1], dt.float32, tag=f"mx{ch}")
                nc.vector.tensor_reduce(mx[:], v[:, sl], mybir.AxisListType.X,
                                        alu.max)
                mxs.append(mx)

            # ---- per-core per-partition [vmax, -vmin] ----
            nc.vector.tensor_tensor(mm[:, 0:1], mxs[0][:], mxs[1][:], alu.max)
            mm_done = nc.vector.tensor_scalar(mm[:, 1:2], mn_prev[:], -1.0,
                                              None, alu.mult)

            # only the DIFFERENCES of the channel means enter the loss, so
            # two dots suffice; the channel-difference subtracts are
            # AllReduce-independent and fill the otherwise idle DVE window
            # (in place: rf1 <- rf0-rf1, rf2 <- rf0-rf2).  The explicit dep
            # on the mm build keeps the Tile scheduler from hoisting them
            # into the pre-phase, which would delay the AllReduce trigger.
            s1 = nc.vector.tensor_tensor(rfs[1][:], rfs[0][:], rfs[1][:],
                                         alu.subtract)
            add_dep_helper(s1.ins, mm_done.ins,
                           reason="keep channel-diff in the AllReduce shadow")
            s2 = nc.vector.tensor_tensor(rfs[2][:], rfs[0][:], rfs[2][:],
                                         alu.subtract)
            add_dep_helper(s2.ins, mm_done.ins,
                           reason="keep channel-diff in the AllReduce shadow")

            if MANUAL_AR:
                # each core broadcasts its [128,2] into its rank's slot of
                # every core's gather buffer, then waits for all 8
                rank = nc.gpsimd.partition_id()
                rdests = [(0, k) for k in range(N_CORES)]
                for r in tc.Switch(rank, N_CORES, hint="ar_slot"):
                    nc.gpsimd.remote_dma_broadcast(
                        gbuf[:, 2 * r:2 * r + 2], mm[:],
                        remote_sem=ar_rsem, local_sem=ar_lsem,
                        rdests=rdests)
                    nc.gpsimd.trigger_dma(count=None)
                # cross-partition reduce of all 8 slots (tracked output).
                # The register-valued semaphore wait (16 = 8 senders x 2
                # remote increments) is attached to this instruction's
                # sync_info so the Tile scheduler cannot hoist it; the
                # scheduling sim treats register waits as satisfiable.
                par = nc.gpsimd.partition_all_reduce(
                    gprr[:], gbuf, channels=P,
                    reduce_op=bass_isa.ReduceOp.max)
                par.wait_op(ar_rsem, ar_ms._reg, "sem-ge")
                # 8 -> 4 -> 2 -> 1 slot max on DVE (tracked gprr input)
                gv = gprr[:].rearrange("p (s two) -> p s two", two=2)
                g4 = pers.tile([P, 8], dt.float32, tag="g4")
                g4v = g4[:].rearrange("p (s two) -> p s two", two=2)
                nc.vector.tensor_tensor(g4v, gv[:, 0:4], gv[:, 4:8], alu.max)
                g2 = pers.tile([P, 4], dt.float32, tag="g2")
                g2v = g2[:].rearrange("p (s two) -> p s two", two=2)
                nc.vector.tensor_tensor(g2v, g4v[:, 0:2], g4v[:, 2:4],
                                        alu.max)
                nc.vector.tensor_tensor(gmm[:], g2[:, 0:2], g2[:, 2:4],
                                        alu.max)
            else:
                flat = pers.tile([1, 2 * P], dt.float32, tag="flat")
                nc.sync.dma_start(flat[:], mm[:])
                l1 = pers.tile([1, 2], dt.float32, tag="l1")
                nc.vector.tensor_reduce(
                    l1[:], flat[:].rearrange("p (c two) -> p two c", two=2),
                    mybir.AxisListType.X, alu.max)
                agi = dram.tile([1, 2], dt.float32)
                aro = dram.tile([1, 2], dt.float32, addr_space="Shared")
                nc.sync.dma_start(agi[:], l1[:])
                nc.gpsimd.collective_compute(
                    "AllReduce", alu.max,
                    replica_groups=[list(range(N_CORES))],
                    ins=[agi.opt()], outs=[aro.opt()])
                nc.sync.dma_start(gmm[:], aro.opt().partition_broadcast(P))

            # debug columns: gmm and mm
            nc.sync.dma_start(out_d[:, 6:8], gmm[:])
            nc.sync.dma_start(out_d[:, 8:10], mm[:])

            # negmin = gmm[:,1:2]; divide by rng in log space (exactly the
            # proven baseline numerics): g = exp(GAMMA*ln(v-imin) + eb),
            # eb = -GAMMA*ln(rng).  rng/eb arithmetic on DVE so ACT only
            # ever touches the Ln/Exp tables.
            negmin = gmm[:, 1:2]
            rng_t = pers.tile([P, 1], dt.float32, tag="rng")
            nc.vector.tensor_tensor(rng_t[:], gmm[:, 0:1], gmm[:, 1:2],
                                    alu.add)
            lnr = pers.tile([P, 1], dt.float32, tag="lnr")
            nc.scalar.activation(lnr[:], rng_t[:],
                                 mybir.ActivationFunctionType.Ln)
            eb = pers.tile([P, 1], dt.float32, tag="eb")
            nc.vector.tensor_scalar(eb[:], lnr[:], -GAMMA, None, alu.mult)

            # ---- post-phase per chunk ----
            for pc, sl in enumerate(POST_SLICES):
                nc.scalar.activation(v[:, sl], v[:, sl],
                                     mybir.ActivationFunctionType.Ln,
                                     bias=negmin, scale=1.0)
            for pc, sl in enumerate(POST_SLICES):
                cw = sl.stop - sl.start
                nc.scalar.activation(v[:, sl], v[:, sl],
                                     mybir.ActivationFunctionType.Exp,
                                     bias=eb[:, 0:1], scale=GAMMA)
                wc = wcp.tile([P, cw], dt.float32, tag="wc",
                              name=f"wc{pc}")
                nc.vector._custom_dve(
                    WSPLITC, out=wc[:], in0=v[:, sl], in1=u[:, sl],
                    s0=SPLIT_S, s1=GAMMA_C, imm2=DELTA_C)
                dmp = wt(cw, tag="dmp")
                for j, c in enumerate((1, 2)):
                    accn = accp.tile([P, 1], dt.float32, tag="accs",
                                     name=f"acc{pc}_{c}")
                    nc.vector.scalar_tensor_tensor(
                        dmp[:], wc[:], 0.0, rfs[c][:, sl],
                        alu.add, alu.mult, accum_out=accn[:])
                    nc.sync.dma_start(
                        out_d[:, (pc * 2 + j):(pc * 2 + j + 1)], accn[:])

    nc.compile()
    return nc


def _get_compiled():
    global _COMPILED
    if _COMPILED is None:
        _COMPILED = _build()
    return _COMPILED


# ---------------------------------------------------------------- entry point

def kernel(reflectance, org_img, seg_label, _trace=False):
    reflectance = np.ascontiguousarray(np.asarray(reflectance, dtype=np.float32))
    org_img = np.asarray(org_img, dtype=np.float32).astype(BF16)
    seg_label = np.asarray(seg_label).astype(BF16)

    nc = _get_compiled()
    in_maps = []
    for i in range(N_CORES):
        in_maps.append({
            "org": np.ascontiguousarray(org_img[i].reshape(C, P, F)),
            "seg": np.ascontiguousarray(seg_label[i].reshape(P, F)),
            "refl": np.ascontiguousarray(reflectance[i].reshape(C, P, F)),
        })
    res = run_bass_kernel_spmd(nc, in_maps, core_ids=list(range(N_CORES)),
                               trace=_trace)
    totals = np.zeros(2, dtype=np.float64)
    for i in range(N_CORES):
        o = res.results[i]["out"].astype(np.float64)   # [128, N_CH x 2 + 6]
        if os.environ.get("KV3_DBG"):
            print(f"core {i}: gmm={o[0, 6:8]} gmm[64]={o[64, 6:8]} "
                  f"mm[0]={o[0, 8:10]} mm[64]={o[64, 8:10]}")
        totals += o[:, :4].sum(axis=0).reshape(N_CH, 2).sum(axis=0)
    # accs hold sum(wc*(r0-r1)) and sum(wc*(r0-r2))
    d01, d02 = totals / float(B * H * W_IMG)
    loss = LOSS_WEIGHT * (d01 ** 2 + d02 ** 2 + (d02 - d01) ** 2)
    if _trace:
        kernel._last_exec_time_ns = res.exec_time_ns
        kernel._last_results = res
    return np.float32(loss)
